# revision 34
# baseline (speedup 1.0000x reference)
"""Trainium2 Bass kernel for ComputeGsct.

Math (per batch b, reduced over N voxels):
    kai(n)   = 10*x2[n,0] - i * x2[n,1]/(OMEGA*EPS0)          (complex scalar)
    A_n      = kai(n) * Gsr_n                                  (complex 3x3)
    C_b      = sum_n A_n @ Grf_n                               (complex 3x3)
    out[b,m,:] = (Re C_b, Im C_b) flattened row-major.

Strategy (v3 — fp16-staged, dual-moment, big-DMA):
  - Batch-parallel sharding: 8 cores x 4 batches each, full N per core.
  - Inputs are cast to fp16 on the host before staging, halving HBM
    traffic. Measured device DMA rate is ~600+ GB/s with >=2 MB
    transfers but per-dma_start overhead dominates small ones, so
    g0/g1 load in q_dma=512-voxel tiles (2.36 MB per transfer) and x2
    loads once per batch; compute runs on q_c=256 sub-chunks.
  - Voxel decomposition is batch-global: v = p*1024 + qq, so every
    tensor tile is a column-slice of the same [128, n/128] layout.
  - The matmul is linear in its stationary operand, so instead of the
    complex product A = kai*Gsr (muls + add/subs) we accumulate two
    real moments  MU = sum kr*gsr(x)grf, MW = sum ki*gsr(x)grf and
    recombine in the host fixup.
  - kai is pre-duplicated x2 on ACT (krx2=[kr,kr], kix2=[ki,ki] per
    voxel); the two DVE muls then run with step-1 fp16 pairs innermost
    (2x_1P) and stride-0 broadcast on a middle AP dim.
  - Per K-chunk group (KGRP=4 chunks of 128 voxels), one TensorE
    matmul: stationary grf [128,72], moving S=[U|W] [128,144],
    accumulated into PSUM [72,144] per batch; diagonal [18,36] blocks
    hold the moments.
"""

import sys

import numpy as np

_TRN_REPO = "/opt/trn_rl_repo"
if _TRN_REPO not in sys.path:
    sys.path.insert(0, _TRN_REPO)

_PAI = 3.141592653589793
_C = 299792458.0
_OMEGA = 2.0 * _PAI * 2.4e9
_MU0 = 4.0 * _PAI * 1e-7
_EPSILON0 = 1.0 / (_C**2 * _MU0)
_KI_SCALE = -1.0 / (_OMEGA * _EPSILON0)

B_FULL, N_FULL = 32, 131072
N_CORES = 8
B_PC = B_FULL // N_CORES  # batches per core
P = 128  # SBUF partitions == matmul contraction size
Q_DMA = 256  # voxels per partition per g0/g1 DMA transfer
Q_C = 256  # voxels per partition per compute sub-chunk
KGRP = 4  # voxel-chunks fused per matmul (diag-block trick)


def build_nc(
    b_pc=B_PC, n=N_FULL, q_dma=Q_DMA, q_c=Q_C, repeat=1, mode="full",
    fused_mul=True, layout="split", io_bufs=6, work_bufs=3, xk_ring="scalar",
):
    """Build the per-core Bass program (SPMD: same program, per-core data).

    repeat>1 wraps the whole computation in a device-side For_i loop; used
    only for benchmarking (wall-time slope over repeat = pure HW time).
    mode ablations:
      "full"  — everything
      "dma"   — loads only (pure DMA floor)
      "kai"   — loads + kai dups (no muls, no matmuls)
      "mul"   — loads + kai dups + DVE muls (no matmuls)
    fused_mul: True = 2 DVE muls with 4-dim APs; False = 18 pair-muls.
    layout: "packed" = host interleaves xk|g0|g1 per (tile, partition) into
    one tensor, one big DMA per tile alternating rings; "split" = three
    separate tensors/DMAs per tile.
    """
    from contextlib import ExitStack

    import concourse.bacc as bacc
    import concourse.mybir as mybir
    from concourse import tile
    from concourse.bass import ts

    f32 = mybir.dt.float32
    f16 = mybir.dt.float16
    FD = 36 * KGRP  # moving cols per matmul group
    SD = 18 * KGRP  # stationary cols per matmul group
    nc = bacc.Bacc("TRN2", target_bir_lowering=False, debug=False)

    tile_v = P * q_dma  # voxels per DMA tile
    assert n % (P * q_dma) == 0 and q_dma % q_c == 0 and q_c % KGRP == 0
    n_tiles = n // (P * q_dma)
    n_sub = q_dma // q_c
    n_grp = q_c // KGRP
    pb = q_dma * 38  # packed per-partition elems: xk(2) + g0(18) + g1(18)

    if layout == "packed":
        xall = nc.dram_tensor(
            "xall", [b_pc, n_tiles, P, pb], f16, kind="ExternalInput"
        )
    else:
        x0 = nc.dram_tensor("x0", [b_pc, n, 9, 2], f16, kind="ExternalInput")
        x1 = nc.dram_tensor("x1", [b_pc, n, 9, 2], f16, kind="ExternalInput")
        x2 = nc.dram_tensor("x2", [b_pc, n, 2], f16, kind="ExternalInput")
    out = nc.dram_tensor("out", [SD, b_pc * FD], f32, kind="ExternalOutput")

    with ExitStack() as ctx:
        tc = ctx.enter_context(tile.TileContext(nc))
        io = ctx.enter_context(tc.tile_pool(name="io", bufs=io_bufs))
        work = ctx.enter_context(tc.tile_pool(name="work", bufs=work_bufs))
        psum = ctx.enter_context(tc.tile_pool(name="psum", bufs=2, space="PSUM"))
        outp = ctx.enter_context(tc.tile_pool(name="outp", bufs=1))

        if repeat > 1:
            loop = ctx.enter_context(tc.For_i(0, repeat, 1))  # noqa: F841

        stage = outp.tile([SD, b_pc * FD], f32)
        if mode != "full":
            nc.vector.memset(stage[:], 0.0)

        for b in range(b_pc):
            if mode == "full":
                ps = psum.tile([SD, FD], f32, tag="ps")

            for t in range(n_tiles):
                # fully-contiguous per-tile loads: voxel v = t*tile_v + p*q_dma
                # + qq, sequential HBM blocks per transfer
                if layout == "packed":
                    big = io.tile([P, pb], f16, tag="big")
                    ring = nc.sync if (b * n_tiles + t) % 2 == 0 else nc.scalar
                    ring.dma_start(big[:], xall[b, t])
                    xk = big[:, 0 : q_dma * 2]
                    g0 = big[:, q_dma * 2 : q_dma * 20]
                    g1 = big[:, q_dma * 20 : q_dma * 38]
                else:
                    xkt = io.tile([P, q_dma * 2], f16, tag="xk")
                    xk_eng = nc.sync if xk_ring == "sync" else nc.scalar
                    xk_eng.dma_start(
                        xkt[:],
                        x2[b, ts(t, tile_v)].rearrange(
                            "(p qq) r -> p (qq r)", p=P
                        ),
                    )
                    g0t = io.tile([P, q_dma * 18], f16, tag="g0")
                    nc.sync.dma_start(
                        g0t[:],
                        x0[b, ts(t, tile_v)].rearrange(
                            "(p qq) m r -> p (qq m r)", p=P
                        ),
                    )
                    g1t = io.tile([P, q_dma * 18], f16, tag="g1")
                    nc.scalar.dma_start(
                        g1t[:],
                        x1[b, ts(t, tile_v)].rearrange(
                            "(p qq) m r -> p (qq m r)", p=P
                        ),
                    )
                    xk, g0, g1 = xkt[:], g0t[:], g1t[:]
                xkbv = xk.rearrange("p (qq r) -> p qq r", r=2)

                if mode == "dma":
                    # consumes on DVE: off every DMA-issue queue, so the
                    # rings stream back-to-back
                    nc.vector.tensor_copy(stage[0:1, 0:18], g0[0:1, 0:18])
                    nc.vector.tensor_copy(stage[0:1, 18:36], g1[0:1, 0:18])
                    nc.vector.tensor_copy(stage[0:1, 36:38], xk[0:1, 0:2])
                    continue

                for s in range(n_sub):
                    qoff = s * q_c
                    # ---- kai, pre-duplicated x2 (ACT, from the batch xk)
                    krx2 = work.tile([P, q_c * 2], f16, tag="krx2")
                    krx2v = krx2[:].rearrange("p (qq u) -> p qq u", u=2)
                    nc.scalar.mul(
                        krx2v,
                        xkbv[:, qoff : qoff + q_c, 0]
                        .unsqueeze(2)
                        .broadcast_to((P, q_c, 2)),
                        10.0,
                    )
                    kix2 = work.tile([P, q_c * 2], f16, tag="kix2")
                    kix2v = kix2[:].rearrange("p (qq u) -> p qq u", u=2)
                    nc.scalar.mul(
                        kix2v,
                        xkbv[:, qoff : qoff + q_c, 1]
                        .unsqueeze(2)
                        .broadcast_to((P, q_c, 2)),
                        _KI_SCALE,
                    )

                    if mode == "kai":
                        nc.vector.tensor_copy(stage[0:1, 38:40], krx2[0:1, 0:2])
                        nc.vector.tensor_copy(stage[0:1, 56:58], kix2[0:1, 0:2])
                        continue

                    # ---- U/W pair-muls (DVE, step-1 fp16 pairs innermost)
                    g0sub = g0[:, s * q_c * 18 : (s + 1) * q_c * 18]
                    S = work.tile([P, q_c * 36], f16, tag="S")
                    if fused_mul:
                        S5 = S[:].rearrange(
                            "p (qq uw m u) -> p qq uw m u", uw=2, m=9, u=2
                        )
                        g04 = g0sub.rearrange(
                            "p (qq m u) -> p qq m u", m=9, u=2
                        )
                        nc.vector.tensor_mul(
                            S5[:, :, 0, :, :],
                            g04,
                            krx2v.unsqueeze(2).broadcast_to((P, q_c, 9, 2)),
                        )
                        nc.vector.tensor_mul(
                            S5[:, :, 1, :, :],
                            g04,
                            kix2v.unsqueeze(2).broadcast_to((P, q_c, 9, 2)),
                        )
                    else:
                        S4 = S[:].rearrange(
                            "p (qq uw c) -> p qq uw c", uw=2, c=18
                        )
                        g0v = g0sub.rearrange("p (qq c) -> p qq c", c=18)
                        for j in range(9):
                            nc.vector.tensor_mul(
                                S4[:, :, 0, 2 * j : 2 * j + 2],
                                g0v[:, :, 2 * j : 2 * j + 2],
                                krx2v,
                            )
                            nc.vector.tensor_mul(
                                S4[:, :, 1, 2 * j : 2 * j + 2],
                                g0v[:, :, 2 * j : 2 * j + 2],
                                kix2v,
                            )

                    if mode == "mul":
                        nc.gpsimd.tensor_copy(stage[0:1, 0:36], S[0:1, 0:36])
                        continue

                    # ---- TensorE: stationary grf [128,72], moving S [128,144]
                    g1v = g1[:, s * q_c * 18 : (s + 1) * q_c * 18].rearrange(
                        "p (g c) -> p g c", c=SD
                    )
                    Sv = S[:].rearrange("p (g c) -> p g c", c=FD)
                    first = t == 0 and s == 0
                    last = t == n_tiles - 1 and s == n_sub - 1
                    for g in range(n_grp):
                        nc.tensor.matmul(
                            ps[:],
                            g1v[:, g, :],
                            Sv[:, g, :],
                            start=(first and g == 0),
                            stop=(last and g == n_grp - 1),
                        )

            if mode == "full":
                nc.scalar.copy(stage[:, b * FD : (b + 1) * FD], ps[:])

        nc.sync.dma_start(out[:], stage[:])

    nc.compile()
    return nc


_NC_CACHE = {}


def _get_nc():
    if "nc" not in _NC_CACHE:
        _NC_CACHE["nc"] = build_nc()
    return _NC_CACHE["nc"]


def fixup(Pm):
    """[Bt, 72, 144] grouped moments -> [Bt, 9, 2] complex C entries.

    PSUM row i = gi*18 + cG (grf component, interleaved re/im), col
    j = gj*36 + uw*18 + cA (uw=0 -> kr-moment, 1 -> ki-moment; cA = gsr
    component). Diagonal gi==gj blocks hold
        M[uw, cG, cA] = sum_v kai_uw[v] * grf[v, cG] * gsr[v, cA].
    """
    Bt = Pm.shape[0]
    M = np.zeros((Bt, 18, 36), Pm.dtype)
    for g in range(KGRP):
        M += Pm[:, 18 * g : 18 * g + 18, 36 * g : 36 * g + 36]
    # -> [Bt, cA, cG]
    MU = M[:, :, 0:18].transpose(0, 2, 1)
    MW = M[:, :, 18:36].transpose(0, 2, 1)
    ii, kk = np.mgrid[0:3, 0:3]
    cr = np.zeros((Bt, 3, 3), np.float32)
    ci = np.zeros((Bt, 3, 3), np.float32)
    for j in range(3):
        ae = 2 * (3 * ii + j)  # gsr component (cA), real part
        be = 2 * (3 * j + kk)  # grf component (cG), real part
        # A_r = u_r - w_i ; A_i = u_i + w_r
        # C_r = A_r@g_r - A_i@g_i ; C_i = A_i@g_r + A_r@g_i
        cr += MU[:, ae, be] - MW[:, ae + 1, be] - MU[:, ae + 1, be + 1] - MW[:, ae, be + 1]
        ci += MU[:, ae + 1, be] + MW[:, ae, be] + MU[:, ae, be + 1] - MW[:, ae + 1, be + 1]
    return np.stack([cr.reshape(Bt, 9), ci.reshape(Bt, 9)], axis=-1)


def prep_inputs(x0, x1, x2, layout="split", q_dma=Q_DMA):
    """fp16-cast and (for the packed layout) interleave xk|g0|g1 per
    (tile, partition) so each device tile is one contiguous DMA."""
    x0 = np.asarray(x0, dtype=np.float16)
    x1 = np.asarray(x1, dtype=np.float16)
    x2 = np.asarray(x2, dtype=np.float16)
    B = x0.shape[0]
    if layout != "packed":
        return {"x0": x0, "x1": x1, "x2": x2}
    nt = N_FULL // (P * q_dma)
    xk = x2.reshape(B, nt, P, q_dma * 2)
    g0 = x0.reshape(B, nt, P, q_dma * 18)
    g1 = x1.reshape(B, nt, P, q_dma * 18)
    return {"xall": np.concatenate([xk, g0, g1], axis=3)}


def run(x0, x1, x2, trace=False):
    from concourse.bass_utils import run_bass_kernel_spmd

    assert np.asarray(x0).shape == (B_FULL, N_FULL, 9, 2)
    full = prep_inputs(x0, x1, x2)

    nc = _get_nc()
    in_maps = [
        {
            k: np.ascontiguousarray(v[i * B_PC : (i + 1) * B_PC])
            for k, v in full.items()
        }
        for i in range(N_CORES)
    ]
    res = run_bass_kernel_spmd(
        nc, in_maps, core_ids=list(range(N_CORES)), trace=trace
    )
    FD = 36 * KGRP
    SD = 18 * KGRP
    Pm = np.concatenate(
        [
            res.results[i]["out"].reshape(SD, B_PC, FD).transpose(1, 0, 2)
            for i in range(N_CORES)
        ],
        axis=0,
    )
    return fixup(Pm), res


def kernel(x0, x1, x2):
    out, _ = run(x0, x1, x2, trace=False)
    return out


def _make_sharded_fn(nc, n_cores=N_CORES, donate=False, repeat=1):
    """Mirror bass2jax.run_bass_via_pjrt's multi-core lowering, returning a
    reusable jitted callable plus metadata, so we can time repeated runs on
    persistent device buffers."""
    import jax
    import jax.core
    from jax.experimental.shard_map import shard_map
    from jax.sharding import Mesh, PartitionSpec

    from concourse import bass2jax, mybir

    bass2jax.install_neuronx_cc_hook()

    partition_name = (
        nc.partition_id_tensor.name if nc.partition_id_tensor else None
    )
    in_names, out_names, out_avals, zero_outs = [], [], [], []
    for alloc in nc.m.functions[0].allocations:
        if not isinstance(alloc, mybir.MemoryLocationSet):
            continue
        name = alloc.memorylocations[0].name
        if alloc.kind == "ExternalInput":
            if name != partition_name:
                in_names.append(name)
        elif alloc.kind == "ExternalOutput":
            shape = tuple(alloc.tensor_shape)
            dtype = mybir.dt.np(alloc.dtype)
            out_names.append(name)
            out_avals.append(jax.core.ShapedArray(shape, dtype))
            zero_outs.append(np.zeros(shape, dtype))
    n_params = len(in_names)
    all_in_names = list(in_names) + list(out_names)
    if partition_name is not None:
        all_in_names.append(partition_name)

    def _body(*args):
        ins = list(args[:n_params])
        prev_outs = list(args[n_params:])
        # `repeat` chained executions of the same NEFF inside one XLA
        # program: each round's outputs feed the next round's (donated-zero)
        # output operands, which defeats CSE and serializes the rounds, so
        # wall-time slope over `repeat` isolates pure on-device time.
        for _ in range(repeat):
            operands = ins + prev_outs
            if partition_name is not None:
                operands.append(bass2jax.partition_id_tensor())
            prev_outs = list(
                bass2jax._bass_exec_p.bind(
                    *operands,
                    out_avals=tuple(out_avals),
                    in_names=tuple(all_in_names),
                    out_names=tuple(out_names),
                    lowering_input_output_aliases=(),
                    sim_require_finite=True,
                    sim_require_nnan=True,
                    nc=nc,
                )
            )
        return tuple(prev_outs)

    devices = jax.devices()[:n_cores]
    mesh = Mesh(np.asarray(devices), ("core",))
    in_specs = (PartitionSpec("core"),) * (n_params + len(out_names))
    out_specs = (PartitionSpec("core"),) * len(out_names)
    donate_argnums = (
        tuple(range(n_params, n_params + len(out_names))) if donate else ()
    )
    fn = jax.jit(
        shard_map(
            _body, mesh=mesh, in_specs=in_specs, out_specs=out_specs,
            check_rep=False,
        ),
        donate_argnums=donate_argnums,
        keep_unused=True,
    )
    return fn, mesh, in_names, out_names, zero_outs


def bench(x0, x1, x2, repeats=(4, 68), calls=8, rounds=4, nc=None, mode="full",
          **bkw):
    """Time the NEFF on-device via the repeat-slope method.

    Builds two XLA programs that chain R executions of the same NEFF
    back-to-back on device; per-call dispatch overhead is identical for
    both, so exec_ns = (T(R2) - T(R1)) / (R2 - R1) is pure HW time.
    The two programs are timed in interleaved rounds (min per round) so
    bursty host/device contention cancels out of the slope.
    """
    import time

    import jax
    from jax.sharding import NamedSharding, PartitionSpec

    concat = prep_inputs(
        x0, x1, x2,
        layout=bkw.get("layout", "split"),
        q_dma=bkw.get("q_dma", Q_DMA),
    )

    prepared = {}
    out = None
    for R in repeats:
        nc_r = build_nc(repeat=R, mode=mode, **bkw)
        fn, mesh, in_names, out_names, zero_outs = _make_sharded_fn(nc_r)
        sh = NamedSharding(mesh, PartitionSpec("core"))
        args = [jax.device_put(concat[n], sh) for n in in_names]
        args += [
            jax.device_put(
                np.zeros((N_CORES * z.shape[0], *z.shape[1:]), z.dtype), sh
            )
            for z in zero_outs
        ]
        out = fn(*args)
        jax.block_until_ready(out)  # compile + warm
        prepared[R] = (fn, args)

    best = {R: float("inf") for R in repeats}
    for _ in range(rounds):
        for R in repeats:
            fn, args = prepared[R]
            t0 = time.perf_counter()
            for _ in range(calls):
                out = fn(*args)
            jax.block_until_ready(out)
            best[R] = min(best[R], (time.perf_counter() - t0) / calls)

    rs = sorted(best)
    per_exec = (best[rs[-1]] - best[rs[0]]) / (rs[-1] - rs[0])
    return per_exec * 1e9, {r: f"{v*1e6:.0f}us" for r, v in best.items()}, (
        np.asarray(out[0]) if out is not None else None
    )


# revision 37
# speedup vs baseline: 1.0816x; 1.0816x over previous
"""Trainium2 Bass kernel for ComputeGsct.

Math (per batch b, reduced over N voxels):
    kai(n)   = 10*x2[n,0] - i * x2[n,1]/(OMEGA*EPS0)          (complex scalar)
    A_n      = kai(n) * Gsr_n                                  (complex 3x3)
    C_b      = sum_n A_n @ Grf_n                               (complex 3x3)
    out[b,m,:] = (Re C_b, Im C_b) flattened row-major.

Strategy (v3 — fp16-staged, dual-moment, big-DMA):
  - Batch-parallel sharding: 8 cores x 4 batches each, full N per core.
  - Inputs are cast to fp16 on the host before staging, halving HBM
    traffic. Measured device DMA rate is ~600+ GB/s with >=2 MB
    transfers but per-dma_start overhead dominates small ones, so
    g0/g1 load in q_dma=512-voxel tiles (2.36 MB per transfer) and x2
    loads once per batch; compute runs on q_c=256 sub-chunks.
  - Voxel decomposition is batch-global: v = p*1024 + qq, so every
    tensor tile is a column-slice of the same [128, n/128] layout.
  - The matmul is linear in its stationary operand, so instead of the
    complex product A = kai*Gsr (muls + add/subs) we accumulate two
    real moments  MU = sum kr*gsr(x)grf, MW = sum ki*gsr(x)grf and
    recombine in the host fixup.
  - kai is pre-duplicated x2 on ACT (krx2=[kr,kr], kix2=[ki,ki] per
    voxel); the two DVE muls then run with step-1 fp16 pairs innermost
    (2x_1P) and stride-0 broadcast on a middle AP dim.
  - Per K-chunk group (KGRP=4 chunks of 128 voxels), one TensorE
    matmul: stationary grf [128,72], moving S=[U|W] [128,144],
    accumulated into PSUM [72,144] per batch; diagonal [18,36] blocks
    hold the moments.
"""

import sys

import numpy as np

_TRN_REPO = "/opt/trn_rl_repo"
if _TRN_REPO not in sys.path:
    sys.path.insert(0, _TRN_REPO)

_PAI = 3.141592653589793
_C = 299792458.0
_OMEGA = 2.0 * _PAI * 2.4e9
_MU0 = 4.0 * _PAI * 1e-7
_EPSILON0 = 1.0 / (_C**2 * _MU0)
_KI_SCALE = -1.0 / (_OMEGA * _EPSILON0)

B_FULL, N_FULL = 32, 131072
N_CORES = 8
B_PC = B_FULL // N_CORES  # batches per core
P = 128  # SBUF partitions == matmul contraction size
Q_DMA = 256  # voxels per partition per g0/g1 DMA transfer
Q_C = 256  # voxels per partition per compute sub-chunk
KGRP = 4  # voxel-chunks fused per matmul (diag-block trick)


def build_nc(
    b_pc=B_PC, n=N_FULL, q_dma=Q_DMA, q_c=Q_C, repeat=1, mode="full",
    fused_mul=True, layout="split", io_bufs=6, work_bufs=4, xk_ring="scalar",
    g1_ring="scalar",
):
    """Build the per-core Bass program (SPMD: same program, per-core data).

    repeat>1 wraps the whole computation in a device-side For_i loop; used
    only for benchmarking (wall-time slope over repeat = pure HW time).
    mode ablations:
      "full"  — everything
      "dma"   — loads only (pure DMA floor)
      "kai"   — loads + kai dups (no muls, no matmuls)
      "mul"   — loads + kai dups + DVE muls (no matmuls)
    fused_mul: True = 2 DVE muls with 4-dim APs; False = 18 pair-muls.
    layout: "packed" = host interleaves xk|g0|g1 per (tile, partition) into
    one tensor, one big DMA per tile alternating rings; "split" = three
    separate tensors/DMAs per tile.
    """
    from contextlib import ExitStack

    import concourse.bacc as bacc
    import concourse.mybir as mybir
    from concourse import tile
    from concourse.bass import ts

    f32 = mybir.dt.float32
    f16 = mybir.dt.float16
    FD = 36 * KGRP  # moving cols per matmul group
    SD = 18 * KGRP  # stationary cols per matmul group
    nc = bacc.Bacc("TRN2", target_bir_lowering=False, debug=False)

    tile_v = P * q_dma  # voxels per DMA tile
    assert n % (P * q_dma) == 0 and q_dma % q_c == 0 and q_c % KGRP == 0
    n_tiles = n // (P * q_dma)
    n_sub = q_dma // q_c
    n_grp = q_c // KGRP
    pb = q_dma * 38  # packed per-partition elems: xk(2) + g0(18) + g1(18)

    if layout == "packed":
        xall = nc.dram_tensor(
            "xall", [b_pc, n_tiles, P, pb], f16, kind="ExternalInput"
        )
    else:
        x0 = nc.dram_tensor("x0", [b_pc, n, 9, 2], f16, kind="ExternalInput")
        x1 = nc.dram_tensor("x1", [b_pc, n, 9, 2], f16, kind="ExternalInput")
        x2 = nc.dram_tensor("x2", [b_pc, n, 2], f16, kind="ExternalInput")
    out = nc.dram_tensor("out", [SD, b_pc * FD], f32, kind="ExternalOutput")

    with ExitStack() as ctx:
        tc = ctx.enter_context(tile.TileContext(nc))
        io = ctx.enter_context(tc.tile_pool(name="io", bufs=io_bufs))
        work = ctx.enter_context(tc.tile_pool(name="work", bufs=work_bufs))
        psum = ctx.enter_context(tc.tile_pool(name="psum", bufs=2, space="PSUM"))
        outp = ctx.enter_context(tc.tile_pool(name="outp", bufs=1))

        if repeat > 1:
            loop = ctx.enter_context(tc.For_i(0, repeat, 1))  # noqa: F841

        stage = outp.tile([SD, b_pc * FD], f32)
        if mode != "full":
            nc.vector.memset(stage[:], 0.0)

        for b in range(b_pc):
            if mode == "full":
                ps = psum.tile([SD, FD], f32, tag="ps")

            for t in range(n_tiles):
                # fully-contiguous per-tile loads: voxel v = t*tile_v + p*q_dma
                # + qq, sequential HBM blocks per transfer
                if layout == "packed":
                    big = io.tile([P, pb], f16, tag="big")
                    ring = nc.sync if (b * n_tiles + t) % 2 == 0 else nc.scalar
                    ring.dma_start(big[:], xall[b, t])
                    xk = big[:, 0 : q_dma * 2]
                    g0 = big[:, q_dma * 2 : q_dma * 20]
                    g1 = big[:, q_dma * 20 : q_dma * 38]
                else:
                    xkt = io.tile([P, q_dma * 2], f16, tag="xk")
                    xk_eng = nc.sync if xk_ring == "sync" else nc.scalar
                    xk_eng.dma_start(
                        xkt[:],
                        x2[b, ts(t, tile_v)].rearrange(
                            "(p qq) r -> p (qq r)", p=P
                        ),
                    )
                    g0t = io.tile([P, q_dma * 18], f16, tag="g0")
                    nc.sync.dma_start(
                        g0t[:],
                        x0[b, ts(t, tile_v)].rearrange(
                            "(p qq) m r -> p (qq m r)", p=P
                        ),
                    )
                    g1t = io.tile([P, q_dma * 18], f16, tag="g1")
                    g1_eng = nc.sync if g1_ring == "sync" else nc.scalar
                    g1_eng.dma_start(
                        g1t[:],
                        x1[b, ts(t, tile_v)].rearrange(
                            "(p qq) m r -> p (qq m r)", p=P
                        ),
                    )
                    xk, g0, g1 = xkt[:], g0t[:], g1t[:]
                xkbv = xk.rearrange("p (qq r) -> p qq r", r=2)

                if mode == "dma":
                    # consumes on DVE: off every DMA-issue queue, so the
                    # rings stream back-to-back
                    nc.vector.tensor_copy(stage[0:1, 0:18], g0[0:1, 0:18])
                    nc.vector.tensor_copy(stage[0:1, 18:36], g1[0:1, 0:18])
                    nc.vector.tensor_copy(stage[0:1, 36:38], xk[0:1, 0:2])
                    continue

                for s in range(n_sub):
                    qoff = s * q_c
                    # ---- kai, pre-duplicated x2 (ACT, from the batch xk)
                    krx2 = work.tile([P, q_c * 2], f16, tag="krx2")
                    krx2v = krx2[:].rearrange("p (qq u) -> p qq u", u=2)
                    nc.scalar.mul(
                        krx2v,
                        xkbv[:, qoff : qoff + q_c, 0]
                        .unsqueeze(2)
                        .broadcast_to((P, q_c, 2)),
                        10.0,
                    )
                    kix2 = work.tile([P, q_c * 2], f16, tag="kix2")
                    kix2v = kix2[:].rearrange("p (qq u) -> p qq u", u=2)
                    nc.scalar.mul(
                        kix2v,
                        xkbv[:, qoff : qoff + q_c, 1]
                        .unsqueeze(2)
                        .broadcast_to((P, q_c, 2)),
                        _KI_SCALE,
                    )

                    if mode == "kai":
                        nc.vector.tensor_copy(stage[0:1, 38:40], krx2[0:1, 0:2])
                        nc.vector.tensor_copy(stage[0:1, 56:58], kix2[0:1, 0:2])
                        continue

                    # ---- U/W pair-muls (DVE, step-1 fp16 pairs innermost)
                    g0sub = g0[:, s * q_c * 18 : (s + 1) * q_c * 18]
                    S = work.tile([P, q_c * 36], f16, tag="S")
                    if fused_mul:
                        S5 = S[:].rearrange(
                            "p (qq uw m u) -> p qq uw m u", uw=2, m=9, u=2
                        )
                        g04 = g0sub.rearrange(
                            "p (qq m u) -> p qq m u", m=9, u=2
                        )
                        nc.vector.tensor_mul(
                            S5[:, :, 0, :, :],
                            g04,
                            krx2v.unsqueeze(2).broadcast_to((P, q_c, 9, 2)),
                        )
                        nc.vector.tensor_mul(
                            S5[:, :, 1, :, :],
                            g04,
                            kix2v.unsqueeze(2).broadcast_to((P, q_c, 9, 2)),
                        )
                    else:
                        S4 = S[:].rearrange(
                            "p (qq uw c) -> p qq uw c", uw=2, c=18
                        )
                        g0v = g0sub.rearrange("p (qq c) -> p qq c", c=18)
                        for j in range(9):
                            nc.vector.tensor_mul(
                                S4[:, :, 0, 2 * j : 2 * j + 2],
                                g0v[:, :, 2 * j : 2 * j + 2],
                                krx2v,
                            )
                            nc.vector.tensor_mul(
                                S4[:, :, 1, 2 * j : 2 * j + 2],
                                g0v[:, :, 2 * j : 2 * j + 2],
                                kix2v,
                            )

                    if mode == "mul":
                        nc.gpsimd.tensor_copy(stage[0:1, 0:36], S[0:1, 0:36])
                        continue

                    # ---- TensorE: stationary grf [128,72], moving S [128,144]
                    g1v = g1[:, s * q_c * 18 : (s + 1) * q_c * 18].rearrange(
                        "p (g c) -> p g c", c=SD
                    )
                    Sv = S[:].rearrange("p (g c) -> p g c", c=FD)
                    first = t == 0 and s == 0
                    last = t == n_tiles - 1 and s == n_sub - 1
                    for g in range(n_grp):
                        nc.tensor.matmul(
                            ps[:],
                            g1v[:, g, :],
                            Sv[:, g, :],
                            start=(first and g == 0),
                            stop=(last and g == n_grp - 1),
                        )

            if mode == "full":
                nc.scalar.copy(stage[:, b * FD : (b + 1) * FD], ps[:])

        nc.sync.dma_start(out[:], stage[:])

    nc.compile()
    return nc


_NC_CACHE = {}


def _get_nc():
    if "nc" not in _NC_CACHE:
        _NC_CACHE["nc"] = build_nc()
    return _NC_CACHE["nc"]


def fixup(Pm):
    """[Bt, 72, 144] grouped moments -> [Bt, 9, 2] complex C entries.

    PSUM row i = gi*18 + cG (grf component, interleaved re/im), col
    j = gj*36 + uw*18 + cA (uw=0 -> kr-moment, 1 -> ki-moment; cA = gsr
    component). Diagonal gi==gj blocks hold
        M[uw, cG, cA] = sum_v kai_uw[v] * grf[v, cG] * gsr[v, cA].
    """
    Bt = Pm.shape[0]
    M = np.zeros((Bt, 18, 36), Pm.dtype)
    for g in range(KGRP):
        M += Pm[:, 18 * g : 18 * g + 18, 36 * g : 36 * g + 36]
    # -> [Bt, cA, cG]
    MU = M[:, :, 0:18].transpose(0, 2, 1)
    MW = M[:, :, 18:36].transpose(0, 2, 1)
    ii, kk = np.mgrid[0:3, 0:3]
    cr = np.zeros((Bt, 3, 3), np.float32)
    ci = np.zeros((Bt, 3, 3), np.float32)
    for j in range(3):
        ae = 2 * (3 * ii + j)  # gsr component (cA), real part
        be = 2 * (3 * j + kk)  # grf component (cG), real part
        # A_r = u_r - w_i ; A_i = u_i + w_r
        # C_r = A_r@g_r - A_i@g_i ; C_i = A_i@g_r + A_r@g_i
        cr += MU[:, ae, be] - MW[:, ae + 1, be] - MU[:, ae + 1, be + 1] - MW[:, ae, be + 1]
        ci += MU[:, ae + 1, be] + MW[:, ae, be] + MU[:, ae, be + 1] - MW[:, ae + 1, be + 1]
    return np.stack([cr.reshape(Bt, 9), ci.reshape(Bt, 9)], axis=-1)


def prep_inputs(x0, x1, x2, layout="split", q_dma=Q_DMA):
    """fp16-cast and (for the packed layout) interleave xk|g0|g1 per
    (tile, partition) so each device tile is one contiguous DMA."""
    x0 = np.asarray(x0, dtype=np.float16)
    x1 = np.asarray(x1, dtype=np.float16)
    x2 = np.asarray(x2, dtype=np.float16)
    B = x0.shape[0]
    if layout != "packed":
        return {"x0": x0, "x1": x1, "x2": x2}
    nt = N_FULL // (P * q_dma)
    xk = x2.reshape(B, nt, P, q_dma * 2)
    g0 = x0.reshape(B, nt, P, q_dma * 18)
    g1 = x1.reshape(B, nt, P, q_dma * 18)
    return {"xall": np.concatenate([xk, g0, g1], axis=3)}


def run(x0, x1, x2, trace=False):
    from concourse.bass_utils import run_bass_kernel_spmd

    assert np.asarray(x0).shape == (B_FULL, N_FULL, 9, 2)
    full = prep_inputs(x0, x1, x2)

    nc = _get_nc()
    in_maps = [
        {
            k: np.ascontiguousarray(v[i * B_PC : (i + 1) * B_PC])
            for k, v in full.items()
        }
        for i in range(N_CORES)
    ]
    res = run_bass_kernel_spmd(
        nc, in_maps, core_ids=list(range(N_CORES)), trace=trace
    )
    FD = 36 * KGRP
    SD = 18 * KGRP
    Pm = np.concatenate(
        [
            res.results[i]["out"].reshape(SD, B_PC, FD).transpose(1, 0, 2)
            for i in range(N_CORES)
        ],
        axis=0,
    )
    return fixup(Pm), res


def kernel(x0, x1, x2):
    out, _ = run(x0, x1, x2, trace=False)
    return out


def _make_sharded_fn(nc, n_cores=N_CORES, donate=False, repeat=1):
    """Mirror bass2jax.run_bass_via_pjrt's multi-core lowering, returning a
    reusable jitted callable plus metadata, so we can time repeated runs on
    persistent device buffers."""
    import jax
    import jax.core
    from jax.experimental.shard_map import shard_map
    from jax.sharding import Mesh, PartitionSpec

    from concourse import bass2jax, mybir

    bass2jax.install_neuronx_cc_hook()

    partition_name = (
        nc.partition_id_tensor.name if nc.partition_id_tensor else None
    )
    in_names, out_names, out_avals, zero_outs = [], [], [], []
    for alloc in nc.m.functions[0].allocations:
        if not isinstance(alloc, mybir.MemoryLocationSet):
            continue
        name = alloc.memorylocations[0].name
        if alloc.kind == "ExternalInput":
            if name != partition_name:
                in_names.append(name)
        elif alloc.kind == "ExternalOutput":
            shape = tuple(alloc.tensor_shape)
            dtype = mybir.dt.np(alloc.dtype)
            out_names.append(name)
            out_avals.append(jax.core.ShapedArray(shape, dtype))
            zero_outs.append(np.zeros(shape, dtype))
    n_params = len(in_names)
    all_in_names = list(in_names) + list(out_names)
    if partition_name is not None:
        all_in_names.append(partition_name)

    def _body(*args):
        ins = list(args[:n_params])
        prev_outs = list(args[n_params:])
        # `repeat` chained executions of the same NEFF inside one XLA
        # program: each round's outputs feed the next round's (donated-zero)
        # output operands, which defeats CSE and serializes the rounds, so
        # wall-time slope over `repeat` isolates pure on-device time.
        for _ in range(repeat):
            operands = ins + prev_outs
            if partition_name is not None:
                operands.append(bass2jax.partition_id_tensor())
            prev_outs = list(
                bass2jax._bass_exec_p.bind(
                    *operands,
                    out_avals=tuple(out_avals),
                    in_names=tuple(all_in_names),
                    out_names=tuple(out_names),
                    lowering_input_output_aliases=(),
                    sim_require_finite=True,
                    sim_require_nnan=True,
                    nc=nc,
                )
            )
        return tuple(prev_outs)

    devices = jax.devices()[:n_cores]
    mesh = Mesh(np.asarray(devices), ("core",))
    in_specs = (PartitionSpec("core"),) * (n_params + len(out_names))
    out_specs = (PartitionSpec("core"),) * len(out_names)
    donate_argnums = (
        tuple(range(n_params, n_params + len(out_names))) if donate else ()
    )
    fn = jax.jit(
        shard_map(
            _body, mesh=mesh, in_specs=in_specs, out_specs=out_specs,
            check_rep=False,
        ),
        donate_argnums=donate_argnums,
        keep_unused=True,
    )
    return fn, mesh, in_names, out_names, zero_outs


def bench(x0, x1, x2, repeats=(4, 68), calls=8, rounds=4, nc=None, mode="full",
          **bkw):
    """Time the NEFF on-device via the repeat-slope method.

    Builds two XLA programs that chain R executions of the same NEFF
    back-to-back on device; per-call dispatch overhead is identical for
    both, so exec_ns = (T(R2) - T(R1)) / (R2 - R1) is pure HW time.
    The two programs are timed in interleaved rounds (min per round) so
    bursty host/device contention cancels out of the slope.
    """
    import time

    import jax
    from jax.sharding import NamedSharding, PartitionSpec

    concat = prep_inputs(
        x0, x1, x2,
        layout=bkw.get("layout", "split"),
        q_dma=bkw.get("q_dma", Q_DMA),
    )

    prepared = {}
    out = None
    for R in repeats:
        nc_r = build_nc(repeat=R, mode=mode, **bkw)
        fn, mesh, in_names, out_names, zero_outs = _make_sharded_fn(nc_r)
        sh = NamedSharding(mesh, PartitionSpec("core"))
        args = [jax.device_put(concat[n], sh) for n in in_names]
        args += [
            jax.device_put(
                np.zeros((N_CORES * z.shape[0], *z.shape[1:]), z.dtype), sh
            )
            for z in zero_outs
        ]
        out = fn(*args)
        jax.block_until_ready(out)  # compile + warm
        prepared[R] = (fn, args)

    best = {R: float("inf") for R in repeats}
    for _ in range(rounds):
        for R in repeats:
            fn, args = prepared[R]
            t0 = time.perf_counter()
            for _ in range(calls):
                out = fn(*args)
            jax.block_until_ready(out)
            best[R] = min(best[R], (time.perf_counter() - t0) / calls)

    rs = sorted(best)
    per_exec = (best[rs[-1]] - best[rs[0]]) / (rs[-1] - rs[0])
    return per_exec * 1e9, {r: f"{v*1e6:.0f}us" for r, v in best.items()}, (
        np.asarray(out[0]) if out is not None else None
    )
